# revision 10
# baseline (speedup 1.0000x reference)
"""ChebConv kernel for 8 trn2 NeuronCores.

Math: the reference's reshape chain collapses exactly to
    X3  = x.reshape(6144, 512)                  (pure reinterpretation)
    A   = densify(COO(filter_rows, filter_cols, filter_vals))   # [2048, 6144]
    Z   = A @ X3                                # [2048, 512]
    W2  = weight.reshape(64, 192)
    YY  = Z.reshape(16384, 64) @ W2             # [16384, 192]
    out = (YY + tile(bias, 3)).reshape(49152, 64)

Sharding: row-parallel over A. Core q takes A rows [256q, 256(q+1)) and
produces out rows [6144q, 6144(q+1)) exactly; unshard is a concat.

Device kernel per core:
  GEMM1: Zt[64s+m, i] = sum_a X3[a, 64s+m] * A_q[i, a]
         out = lhsT.T @ rhs with lhsT = X3 k-chunks [128, 128m], rhs = A_q^T
         k-chunks [128, 256], accumulated over 48 k-chunks into 4 PSUM tiles.
  GEMM2: per s-block: YY[8i+s, j] = sum_m Zt[64s+m, i] * W2[m, j]
         lhsT = Zt slice [64, 128] (base partition 0 or 64), rhs = W2 copy on
         the same partition half, K=64 matmul via PE row-group addressing.
  Bias add fused into the PSUM drain on DVE; output rows land contiguous.
"""

import numpy as np

import concourse.bass as bass
import concourse.mybir as mybir
import concourse.tile as tile
from concourse.bass_utils import run_bass_kernel_spmd

N_CORES = 8
NV = 2048            # A rows (vertices)
KA = 6144            # contraction dim (rows of X3)
NX = 512             # X3 cols
RQ = NV // N_CORES   # 256 A-rows per core
KC = KA // 128       # 48 k-chunks
SUPER = 12           # k-chunks per DMA super-chunk
NSUP = KC // SUPER   # 8 super-chunks
F32 = mybir.dt.float32

_CACHE = {}


def _build_bass():
    nc = bass.Bass()
    xa_d = nc.dram_tensor("xa", [KC, 128, NX + RQ], F32, kind="ExternalInput")
    c_d = nc.dram_tensor("consts", [128, 384], F32, kind="ExternalInput")
    y_d = nc.dram_tensor("y", [NV, 192], F32, kind="ExternalOutput")

    with tile.TileContext(nc) as tc:
        with (
            tc.tile_pool(name="consts", bufs=1) as cpool,
            tc.tile_pool(name="xin", bufs=NSUP) as xpool,
            tc.tile_pool(name="zt", bufs=1) as zpool,
            tc.tile_pool(name="outp", bufs=2) as opool,
            tc.tile_pool(name="zpsum", bufs=1, space="PSUM") as zppool,
            tc.tile_pool(name="ypsum", bufs=2, space="PSUM") as yppool,
        ):
            cst = cpool.tile([128, 384], F32)
            nc.sync.dma_start(cst[:], c_d[:])
            w2_sb = cst[:, 0:192]
            bias_sb = cst[:, 192:384]

            # DVE touches the const tile early so later DVE ops don't
            # need a second (DMA-lane) sync wait.
            scr1 = cpool.tile([128, 1], F32, name="scr1")
            nc.vector.tensor_copy(out=scr1[:], in_=cst[:, :1])

            # PE touches w2_sb early so later GEMM2 matmuls only wait on
            # the DVE drain sem (walrus allows 1 sync wait per Matmult).
            warm = yppool.tile([128, 192], F32, name="warm", tag="yp")
            nc.tensor.matmul(warm[:], lhsT=w2_sb[:, :128], rhs=w2_sb,
                             start=True, stop=True)

            # --- GEMM1: Zt [512, 256] in 4 PSUM tiles of [128, 256] -----
            zps = [zppool.tile([128, RQ], F32, name=f"zp{m}", tag=f"zp{m}")
                   for m in range(4)]
            for sc in range(NSUP):
                eng = nc.sync if sc % 2 == 0 else nc.scalar
                xa = xpool.tile([128, SUPER, NX + RQ], F32)
                eng.dma_start(xa[:], xa_d[sc * SUPER:(sc + 1) * SUPER]
                              .rearrange("k p n -> p k n"))
                for kk in range(SUPER):
                    kc = sc * SUPER + kk
                    for m in range(4):
                        nc.tensor.matmul(
                            zps[m][:],
                            lhsT=xa[:, kk, 128 * m:128 * (m + 1)],
                            rhs=xa[:, kk, NX:NX + RQ],
                            start=(kc == 0),
                            stop=(kc == KC - 1),
                        )

            # drain PSUM -> SBUF (same partitions)
            zsb = [zpool.tile([128, RQ], F32, name=f"zs{m}", tag=f"zs{m}")
                   for m in range(4)]
            for m in range(4):
                nc.vector.tensor_copy(out=zsb[m][:], in_=zps[m][:])

            # --- GEMM2 + bias + store ----------------------------------
            # lhsT for (s, ic2): zsb[s//2][64*(s%2) + m, 128*ic2 + l]
            # out rows y[1024*ic2 + 8l + s] contiguous per ic2 block.
            for ic2 in range(2):
                osb = opool.tile([128, 8, 192], F32)
                for s in range(8):
                    half = 64 * (s % 2)
                    yp = yppool.tile([128, 192], F32, name="yp", tag="yp")
                    nc.tensor.matmul(
                        yp[:],
                        lhsT=zsb[s // 2][half:half + 64,
                                         128 * ic2:128 * (ic2 + 1)],
                        rhs=w2_sb[half:half + 64, :192],
                        start=True,
                        stop=True,
                    )
                    nc.vector.tensor_add(
                        out=osb[:, s, :], in0=yp[:], in1=bias_sb)
                nc.sync.dma_start(
                    y_d[1024 * ic2:1024 * (ic2 + 1)]
                    .rearrange("(l s) j -> l (s j)", s=8),
                    osb[:].rearrange("p s j -> p (s j)"),
                )
    _split_multi_waits(nc)
    return nc


def _split_multi_waits(nc):
    """walrus codegen accepts only one sync wait per instruction in this
    toolchain; hoist extra waits onto standalone EventSemaphore ops placed
    immediately before, on the same engine."""
    n = [0]
    for bb in nc.main_func.blocks:
        new_insts = []
        for ins in bb.instructions:
            si = ins.sync_info
            if si is not None and si.on_wait and len(si.on_wait) > 1:
                extra, si.on_wait = si.on_wait[1:], si.on_wait[:1]
                for w in extra:
                    n[0] += 1
                    ev = mybir.InstEventSemaphore(
                        name=f"{ins.name}-sw{n[0]}",
                        engine=ins.engine,
                        ins=[],
                        outs=[],
                        sync_info=mybir.SyncInfo(on_wait=[w], on_update=[]),
                    )
                    nc.register_instruction(ev, overwrite=True)
                    new_insts.append(ev)
            new_insts.append(ins)
        bb.instructions[:] = new_insts


def _prep_inputs(x, weight, bias, filter_vals, filter_rows, filter_cols):
    x = np.ascontiguousarray(np.asarray(x, dtype=np.float32))
    weight = np.asarray(weight, dtype=np.float32)
    bias = np.asarray(bias, dtype=np.float32)
    fv = np.asarray(filter_vals, dtype=np.float32)
    fr = np.asarray(filter_rows).astype(np.int64)
    fc = np.asarray(filter_cols).astype(np.int64)

    x3 = x.reshape(KC, 128, NX)

    a_dense = np.zeros((NV, KA), dtype=np.float32)
    np.add.at(a_dense, (fr, fc), fv)

    w2 = weight.reshape(64, 192)
    w2dup = np.concatenate([w2, w2], axis=0)                    # [128, 192]
    bias3 = np.tile(bias, 3)                                    # [192]
    biasr = np.broadcast_to(bias3, (128, 192))
    consts = np.ascontiguousarray(
        np.concatenate([w2dup, biasr], axis=1), dtype=np.float32)  # [128, 384]

    in_maps = []
    for q in range(N_CORES):
        at_q = a_dense[RQ * q:RQ * (q + 1), :].T.reshape(KC, 128, RQ)
        xa_q = np.concatenate([x3, at_q], axis=2)   # [48, 128, 768]
        in_maps.append({"xa": xa_q, "consts": consts})
    return in_maps


def kernel(x, weight, bias, filter_vals, filter_rows, filter_cols,
           _trace=False, _trace_kwargs=None):
    if "nc" not in _CACHE:
        _CACHE["nc"] = _build_bass()
    nc = _CACHE["nc"]

    in_maps = _prep_inputs(x, weight, bias, filter_vals,
                           filter_rows, filter_cols)
    res = run_bass_kernel_spmd(
        nc, in_maps, core_ids=list(range(N_CORES)),
        trace=_trace, **(_trace_kwargs or {}))
    out = np.concatenate(
        [res.results[q]["y"].reshape(RQ * 24, 64) for q in range(N_CORES)],
        axis=0)
    if _trace:
        _CACHE["last_results"] = res
    return out


# revision 13
# speedup vs baseline: 1.4986x; 1.4986x over previous
"""ChebConv kernel for 8 trn2 NeuronCores.

Math: the reference's reshape chain collapses exactly to
    X3  = x.reshape(6144, 512)                  (pure reinterpretation)
    A   = densify(COO(filter_rows, filter_cols, filter_vals))   # [2048, 6144]
    Z   = A @ X3                                # [2048, 512]
    W2  = weight.reshape(64, 192)
    YY  = Z.reshape(16384, 64) @ W2             # [16384, 192]
    out = (YY + tile(bias, 3)).reshape(49152, 64)

Sharding: row-parallel over A. Core q takes A rows [256q, 256(q+1)) and
produces out rows [6144q, 6144(q+1)) exactly; unshard is a concat.

Device kernel per core:
  GEMM1: Zt[64s+m, i] = sum_a X3[a, 64s+m] * A_q[i, a]
         out = lhsT.T @ rhs with lhsT = X3 k-chunks [128, 128m], rhs = A_q^T
         k-chunks [128, 256], accumulated over 48 k-chunks into 4 PSUM tiles.
  GEMM2: per s-block: YY[8i+s, j] = sum_m Zt[64s+m, i] * W2[m, j]
         lhsT = Zt slice [64, 128] (base partition 0 or 64), rhs = W2 copy on
         the same partition half, K=64 matmul via PE row-group addressing.
  Bias add fused into the PSUM drain on DVE; output rows land contiguous.
"""

import numpy as np

import concourse.bass as bass
import concourse.mybir as mybir
import concourse.tile as tile
from concourse.bass_utils import run_bass_kernel_spmd

N_CORES = 8
NV = 2048            # A rows (vertices)
KA = 6144            # contraction dim (rows of X3)
NX = 512             # X3 cols
RQ = NV // N_CORES   # 256 A-rows per core
KC = KA // 128       # 48 k-chunks
SUPER = 1            # k-chunks per DMA super-chunk
NSUP = KC // SUPER   # 8 super-chunks
F32 = mybir.dt.float32

_CACHE = {}


def _build_bass():
    nc = bass.Bass()
    xa_d = nc.dram_tensor("xa", [KC, 128, NX + RQ], F32, kind="ExternalInput")
    c_d = nc.dram_tensor("consts", [128, 384], F32, kind="ExternalInput")
    y_d = nc.dram_tensor("y", [NV, 192], F32, kind="ExternalOutput")

    with tile.TileContext(nc) as tc:
        with (
            tc.tile_pool(name="consts", bufs=1) as cpool,
            tc.tile_pool(name="xin", bufs=NSUP) as xpool,
            tc.tile_pool(name="zt", bufs=1) as zpool,
            tc.tile_pool(name="outp", bufs=2) as opool,
            tc.tile_pool(name="zpsum", bufs=1, space="PSUM") as zppool,
            tc.tile_pool(name="ypsum", bufs=2, space="PSUM") as yppool,
        ):
            cst = cpool.tile([128, 384], F32)
            nc.sync.dma_start(cst[:], c_d[:])
            w2_sb = cst[:, 0:192]
            bias_sb = cst[:, 192:384]

            # DVE touches the const tile early so later DVE ops don't
            # need a second (DMA-lane) sync wait.
            scr1 = cpool.tile([128, 1], F32, name="scr1")
            nc.vector.tensor_copy(out=scr1[:], in_=cst[:, :1])

            # PE touches w2_sb early so later GEMM2 matmuls only wait on
            # the DVE drain sem (walrus allows 1 sync wait per Matmult).
            warm = yppool.tile([128, 192], F32, name="warm", tag="yp")
            nc.tensor.matmul(warm[:], lhsT=w2_sb[:, :128], rhs=w2_sb,
                             start=True, stop=True)

            # --- GEMM1: Zt [512, 256] in 4 PSUM tiles of [128, 256] -----
            zps = [zppool.tile([128, RQ], F32, name=f"zp{m}", tag=f"zp{m}")
                   for m in range(4)]
            for sc in range(NSUP):
                eng = nc.sync if sc % 2 == 0 else nc.scalar
                xa = xpool.tile([128, SUPER, NX + RQ], F32)
                eng.dma_start(xa[:], xa_d[sc * SUPER:(sc + 1) * SUPER]
                              .rearrange("k p n -> p k n"))
                for kk in range(SUPER):
                    kc = sc * SUPER + kk
                    for m in range(4):
                        nc.tensor.matmul(
                            zps[m][:],
                            lhsT=xa[:, kk, 128 * m:128 * (m + 1)],
                            rhs=xa[:, kk, NX:NX + RQ],
                            start=(kc == 0),
                            stop=(kc == KC - 1),
                        )

            # drain PSUM -> SBUF (same partitions)
            zsb = [zpool.tile([128, RQ], F32, name=f"zs{m}", tag=f"zs{m}")
                   for m in range(4)]
            for m in range(4):
                nc.vector.tensor_copy(out=zsb[m][:], in_=zps[m][:])

            # --- GEMM2 + bias + store ----------------------------------
            # lhsT for (s, ic2): zsb[s//2][64*(s%2) + m, 128*ic2 + l]
            # out rows y[1024*ic2 + 8l + s] contiguous per ic2 block.
            for ic2 in range(2):
                osb = opool.tile([128, 8, 192], F32)
                for s in range(8):
                    half = 64 * (s % 2)
                    yp = yppool.tile([128, 192], F32, name="yp", tag="yp")
                    nc.tensor.matmul(
                        yp[:],
                        lhsT=zsb[s // 2][half:half + 64,
                                         128 * ic2:128 * (ic2 + 1)],
                        rhs=w2_sb[half:half + 64, :192],
                        start=True,
                        stop=True,
                    )
                    nc.vector.tensor_add(
                        out=osb[:, s, :], in0=yp[:], in1=bias_sb)
                nc.sync.dma_start(
                    y_d[1024 * ic2:1024 * (ic2 + 1)]
                    .rearrange("(l s) j -> l (s j)", s=8),
                    osb[:].rearrange("p s j -> p (s j)"),
                )
    _split_multi_waits(nc)
    return nc


def _split_multi_waits(nc):
    """walrus codegen accepts only one sync wait per instruction in this
    toolchain; hoist extra waits onto standalone EventSemaphore ops placed
    immediately before, on the same engine."""
    n = [0]
    for bb in nc.main_func.blocks:
        new_insts = []
        for ins in bb.instructions:
            si = ins.sync_info
            if si is not None and si.on_wait and len(si.on_wait) > 1:
                extra, si.on_wait = si.on_wait[1:], si.on_wait[:1]
                for w in extra:
                    n[0] += 1
                    ev = mybir.InstEventSemaphore(
                        name=f"{ins.name}-sw{n[0]}",
                        engine=ins.engine,
                        ins=[],
                        outs=[],
                        sync_info=mybir.SyncInfo(on_wait=[w], on_update=[]),
                    )
                    nc.register_instruction(ev, overwrite=True)
                    new_insts.append(ev)
            new_insts.append(ins)
        bb.instructions[:] = new_insts


def _prep_inputs(x, weight, bias, filter_vals, filter_rows, filter_cols):
    x = np.ascontiguousarray(np.asarray(x, dtype=np.float32))
    weight = np.asarray(weight, dtype=np.float32)
    bias = np.asarray(bias, dtype=np.float32)
    fv = np.asarray(filter_vals, dtype=np.float32)
    fr = np.asarray(filter_rows).astype(np.int64)
    fc = np.asarray(filter_cols).astype(np.int64)

    x3 = x.reshape(KC, 128, NX)

    a_dense = np.zeros((NV, KA), dtype=np.float32)
    np.add.at(a_dense, (fr, fc), fv)

    w2 = weight.reshape(64, 192)
    w2dup = np.concatenate([w2, w2], axis=0)                    # [128, 192]
    bias3 = np.tile(bias, 3)                                    # [192]
    biasr = np.broadcast_to(bias3, (128, 192))
    consts = np.ascontiguousarray(
        np.concatenate([w2dup, biasr], axis=1), dtype=np.float32)  # [128, 384]

    in_maps = []
    for q in range(N_CORES):
        at_q = a_dense[RQ * q:RQ * (q + 1), :].T.reshape(KC, 128, RQ)
        xa_q = np.concatenate([x3, at_q], axis=2)   # [48, 128, 768]
        in_maps.append({"xa": xa_q, "consts": consts})
    return in_maps


def kernel(x, weight, bias, filter_vals, filter_rows, filter_cols,
           _trace=False, _trace_kwargs=None):
    if "nc" not in _CACHE:
        _CACHE["nc"] = _build_bass()
    nc = _CACHE["nc"]

    in_maps = _prep_inputs(x, weight, bias, filter_vals,
                           filter_rows, filter_cols)
    res = run_bass_kernel_spmd(
        nc, in_maps, core_ids=list(range(N_CORES)),
        trace=_trace, **(_trace_kwargs or {}))
    out = np.concatenate(
        [res.results[q]["y"].reshape(RQ * 24, 64) for q in range(N_CORES)],
        axis=0)
    if _trace:
        _CACHE["last_results"] = res
    return out
